# revision 5
# baseline (speedup 1.0000x reference)
"""AdaCoF sampler kernel for 8 TRN2 NeuronCores.

Strategy: shard (batch, y-half) -> 8 cores. Per core, build a DRAM patch
table REP[cell(y,x)] = the 2x2x3ch bilinear corner values (12 fp32, stored
at 256B stride). Per (tap, 64-row block): compute sample coordinates on
DVE, floor via the +3*2^22 magic constant, form int16 cell indices, and
fetch all 12 corner values per tap-pixel with one dma_gather descriptor
(1024-index chunks round-robin over 4 SWDGE queues). Bilinear lerp +
49-tap accumulation on DVE.
"""
import sys, types

sys.path.insert(0, "/opt/trn_rl_repo")
import numpy as np

# -- NTFF profile hook (stub antenv lacks axon_hooks; needed for trace=True) --
if "antenv.axon_hooks" not in sys.modules:
    _m = types.ModuleType("antenv.axon_hooks")
    _m._hook = None
    def _set(h, _m=_m): _m._hook = h
    def _get(_m=_m): return _m._hook
    _m.set_axon_ntff_profile_hook = _set
    _m.get_axon_ntff_profile_hook = _get
    sys.modules["antenv.axon_hooks"] = _m
    import antenv
    antenv.axon_hooks = _m
try:
    from trn_agent_boot.trn_boot import _ntff_profile_via_ctypes
    sys.modules["antenv.axon_hooks"].set_axon_ntff_profile_hook(
        _ntff_profile_via_ctypes("/opt/axon/libaxon_pjrt.so"))
except Exception:
    pass

import concourse.bass as bass
import concourse.bacc as bacc
import concourse.mybir as mybir
from concourse.tile import TileContext
from concourse import bass_utils
from concourse.bass_utils import run_bass_kernel_spmd
from concourse._compat import exact_div

bass_utils.upload_artifacts = lambda tmpdir: tmpdir  # no S3 in container

F32 = mybir.dt.float32
I16 = mybir.dt.int16
ALU = mybir.AluOpType

B, C, H, W = 4, 3, 256, 256
K2 = 49
C1 = W / (W - 1.0)          # 256/255 coordinate scale
MAGIC = 12582912.0          # 3 * 2^22: +/- round-to-int magic for fp32
SLAB = 144                  # image slab rows per core (128 + 2*8 halo)
PAD = 8
BLK = 64                    # output rows per gather block
WIN = BLK + 2 * PAD         # 80 window rows per block
NCELL_BLK = WIN * W         # 20480 cells per block window (< 2^15)
STRIDE = 64                 # fp32 elements per REP row (256 B)
ES = 12                     # payload elements per gather
NI = 1024                   # indices per dma_gather (ring-limited)
NCHUNK = (BLK * W) // NI    # 16 chunks per (tap, block)
FREE = 2 * BLK              # free dim of [128, FREE] coordinate planes


def _dma_gather(gp, out_ap, in_ap, idxs_ap, num_idxs, elem_size, elem_step, queue_num):
    """dma_gather minus the over-strict elem_size%256 assert (the ucode only
    requires the row stride to be a 256B multiple)."""
    stride_bytes = elem_step * mybir.dt.size(in_ap.dtype)
    _in_ap = gp.lower_ap_dma(in_ap, for_custom_bir_dma=True)
    return gp.add_instruction(
        mybir.InstDMAGatherAnt(
            name=gp.bass.get_next_instruction_name(),
            ins=[*_in_ap, gp.lower_ap(idxs_ap), gp.lower_val_access(gp.to_reg(num_idxs))],
            outs=[gp.lower_ap(out_ap)],
            transpose=False, num_idxs=num_idxs, elem_size=elem_size,
            stride_bytes_256=exact_div(stride_bytes, 256), gen_mode=0,
            single_packet=True, queue_num=queue_num,
            sbuf_tokens_per_rank=0, sbuf_free_dim_per_rank=0,
            sbuf_free_dim_pad_per_rank=0, sbuf_byte_offset=0,
        )
    )


def build_program():
    nc = bacc.Bacc(None, target_bir_lowering=False, num_swdge_queues=4)

    img = nc.dram_tensor("img", [C, SLAB, W], F32, kind="ExternalInput")
    offs = nc.dram_tensor("offs", [2 * K2, 2 * BLK, W], F32, kind="ExternalInput")
    wts = nc.dram_tensor("wts", [K2, 2 * BLK, W], F32, kind="ExternalInput")
    ygrid = nc.dram_tensor("ygrid", [2, 128, FREE], F32, kind="ExternalInput")
    xgrid = nc.dram_tensor("xgrid", [128, FREE], F32, kind="ExternalInput")
    cconst = nc.dram_tensor("cconst", [128, 2], F32, kind="ExternalInput")
    out = nc.dram_tensor("out", [C, 2 * BLK, W], F32, kind="ExternalOutput")

    rep = nc.dram_tensor("rep", [SLAB * W, STRIDE], F32, kind="Internal")

    with TileContext(nc) as tc:
        with tc.tile_pool(name="cst", bufs=1) as cpool, \
             tc.tile_pool(name="repb", bufs=2) as rpool, \
             tc.tile_pool(name="strm", bufs=3) as spool, \
             tc.tile_pool(name="crd", bufs=3) as kpool, \
             tc.tile_pool(name="gth", bufs=3) as gpool, \
             tc.tile_pool(name="acc", bufs=1) as apool:

            # ---- constants ----
            xg = cpool.tile([128, FREE], F32)
            nc.sync.dma_start(out=xg[:], in_=xgrid[:])
            yg = cpool.tile([128, 2 * FREE], F32)
            nc.sync.dma_start(out=yg[:, 0:FREE], in_=ygrid[0])
            nc.sync.dma_start(out=yg[:, FREE:2 * FREE], in_=ygrid[1])
            cc = cpool.tile([128, 2], F32)
            nc.sync.dma_start(out=cc[:], in_=cconst[:])

            # wrap tile for gather indices: [128, NCHUNK*64] int16.
            # tx cores (odd 16-partition groups) read real indices; rx cores
            # (even groups) only need non-negative filler -> zero once.
            wrap = cpool.tile([128, NCHUNK * NI // 16], I16)
            nc.gpsimd.memset(wrap[:], 0)

            # ---- phase A: build REP ----
            # cells row-major (slabrow, x); slot layout corner*3+c:
            #   [v00 c0..c2 | v01 c0..c2 | v10 c0..c2 | v11 c0..c2]
            for grp, g0, rows in ((0, 0, 128), (1, 128, SLAB - 128)):
                rsb = rpool.tile([128, W * ES], F32, tag="rsb")
                for c in range(C):
                    it = rpool.tile([128, W], F32, tag="it")
                    its = rpool.tile([128, W], F32, tag="its")
                    nc.sync.dma_start(out=it[:rows], in_=img[c, g0:g0 + rows, :])
                    # +1-row shifted copy (clamped at slab end)
                    hi = min(g0 + rows + 1, SLAB)
                    nc.sync.dma_start(out=its[:hi - g0 - 1], in_=img[c, g0 + 1:hi, :])
                    if hi - g0 - 1 < rows:
                        nc.sync.dma_start(out=its[rows - 1:rows], in_=img[c, SLAB - 1:SLAB, :])
                    rv = rsb[:rows].rearrange("p (x s) -> p x s", s=ES)
                    # v00
                    nc.vector.tensor_copy(out=rv[:, :, c], in_=it[:rows])
                    # v01 (x+1, clamp at 255)
                    nc.vector.tensor_copy(out=rv[:, 0:W - 1, 3 + c], in_=it[:rows, 1:W])
                    nc.vector.tensor_copy(out=rv[:, W - 1:W, 3 + c], in_=it[:rows, W - 1:W])
                    # v10 (y+1)
                    nc.vector.tensor_copy(out=rv[:, :, 6 + c], in_=its[:rows])
                    # v11 (y+1, x+1)
                    nc.vector.tensor_copy(out=rv[:, 0:W - 1, 9 + c], in_=its[:rows, 1:W])
                    nc.vector.tensor_copy(out=rv[:, W - 1:W, 9 + c], in_=its[:rows, W - 1:W])
                for r0 in range(0, rows, 32):
                    r1 = min(r0 + 32, rows)
                    nc.sync.dma_start(
                        out=rep[(g0 + r0) * W:(g0 + r1) * W, 0:ES].rearrange(
                            "(p x) s -> p x s", x=W),
                        in_=rsb[r0:r1])

            # ---- phase B ----
            for blk in range(2):
                accs = [apool.tile([128, FREE], F32, tag=f"acc{c}", name=f"acc{c}_{blk}") for c in range(C)]
                for a in accs:
                    nc.gpsimd.memset(a[:], 0.0)
                ygb = yg[:, blk * FREE:(blk + 1) * FREE]
                for k in range(K2):
                    dx = spool.tile([128, FREE], F32, tag="dx")
                    dy = spool.tile([128, FREE], F32, tag="dy")
                    wt = spool.tile([128, FREE], F32, tag="wt")
                    src = offs[2 * k, blk * BLK:(blk + 1) * BLK, :].rearrange(
                        "y (xh p) -> p (y xh)", p=128)
                    nc.sync.dma_start(out=dx[:], in_=src)
                    src = offs[2 * k + 1, blk * BLK:(blk + 1) * BLK, :].rearrange(
                        "y (xh p) -> p (y xh)", p=128)
                    nc.sync.dma_start(out=dy[:], in_=src)
                    src = wts[k, blk * BLK:(blk + 1) * BLK, :].rearrange(
                        "y (xh p) -> p (y xh)", p=128)
                    nc.scalar.dma_start(out=wt[:], in_=src)

                    # coordinates
                    px = kpool.tile([128, FREE], F32, tag="px")
                    py = kpool.tile([128, FREE], F32, tag="py")
                    nc.vector.tensor_scalar(out=px[:], in0=dx[:], scalar1=C1,
                                            scalar2=None, op0=ALU.mult)
                    nc.vector.tensor_tensor(out=px[:], in0=px[:], in1=xg[:], op=ALU.add)
                    nc.vector.tensor_scalar(out=px[:], in0=px[:], scalar1=0.0,
                                            scalar2=float(W - 1), op0=ALU.max, op1=ALU.min)
                    nc.vector.tensor_scalar(out=py[:], in0=dy[:], scalar1=C1,
                                            scalar2=None, op0=ALU.mult)
                    nc.vector.tensor_tensor(out=py[:], in0=py[:], in1=ygb, op=ALU.add)
                    nc.vector.tensor_scalar(out=py[:], in0=py[:], scalar1=cc[:, 0:1],
                                            scalar2=cc[:, 1:2], op0=ALU.max, op1=ALU.min)
                    # floors via magic round of (v - 0.5)
                    x0 = kpool.tile([128, FREE], F32, tag="x0")
                    y0 = kpool.tile([128, FREE], F32, tag="y0")
                    fx = kpool.tile([128, FREE], F32, tag="fx")
                    av = kpool.tile([128, FREE], F32, tag="av")
                    bv = kpool.tile([128, FREE], F32, tag="bv")
                    nc.vector.tensor_scalar(out=x0[:], in0=px[:], scalar1=-0.5,
                                            scalar2=MAGIC, op0=ALU.add, op1=ALU.add)
                    nc.vector.tensor_scalar(out=x0[:], in0=x0[:], scalar1=MAGIC,
                                            scalar2=None, op0=ALU.subtract)
                    nc.vector.tensor_scalar(out=y0[:], in0=py[:], scalar1=-0.5,
                                            scalar2=MAGIC, op0=ALU.add, op1=ALU.add)
                    nc.vector.tensor_scalar(out=y0[:], in0=y0[:], scalar1=MAGIC,
                                            scalar2=None, op0=ALU.subtract)
                    nc.vector.tensor_tensor(out=fx[:], in0=px[:], in1=x0[:], op=ALU.subtract)
                    # fy -> av = w*fy, bv = w - av  (reuse py as fy)
                    nc.vector.tensor_tensor(out=py[:], in0=py[:], in1=y0[:], op=ALU.subtract)
                    nc.vector.tensor_tensor(out=av[:], in0=wt[:], in1=py[:], op=ALU.mult)
                    nc.vector.tensor_tensor(out=bv[:], in0=wt[:], in1=av[:], op=ALU.subtract)
                    # idx = (y0 - blk*BLK)*256 + x0, int16
                    idx = kpool.tile([128, FREE], I16, tag="idx")
                    nc.vector.tensor_scalar(out=y0[:], in0=y0[:], scalar1=float(W),
                                            scalar2=float(-blk * BLK * W),
                                            op0=ALU.mult, op1=ALU.add)
                    nc.vector.tensor_tensor(out=idx[:], in0=y0[:], in1=x0[:], op=ALU.add)

                    # fold idx [128, FREE] -> wrap odd groups (tx cores of q0..q3)
                    # stream j = y*256 + x ; wrap[pp, cc]: pp=j%16, cc=j//16
                    # src partition p = pd*16+pp, free f = 2y+xh
                    # dst col = y*16 + xh*8 + pd
                    for pd in range(8):
                        s = idx[16 * pd:16 * pd + 16, :].rearrange(
                            "p (y xh) -> p y xh", xh=2)
                        d = wrap[16:32, :].rearrange(
                            "p (y xh pd) -> p y xh pd", xh=2, pd=8)[:, :, :, pd]
                        eng = nc.sync if pd % 2 == 0 else nc.scalar
                        eng.dma_start(out=d, in_=s)
                    for gdst in (3, 5, 7):
                        eng = nc.sync if gdst == 5 else nc.scalar
                        eng.dma_start(out=wrap[16 * gdst:16 * gdst + 16, :],
                                      in_=wrap[16:32, :])

                    # gather: 16 chunks of 1024 idx, queues round-robin
                    gt = gpool.tile([128, 2 * BLK, ES], F32, tag="gt")
                    for r in range(NCHUNK):
                        _dma_gather(
                            nc.gpsimd,
                            out_ap=gt[:, 8 * r:8 * r + 8, :],
                            in_ap=rep[blk * BLK * W:blk * BLK * W + NCELL_BLK, 0:ES],
                            idxs_ap=wrap[:, 64 * r:64 * r + 64],
                            num_idxs=NI, elem_size=ES, elem_step=STRIDE,
                            queue_num=r % 4,
                        )

                    # combine: per channel lerp + accumulate
                    t0 = kpool.tile([128, FREE], F32, tag="t0")
                    t1 = kpool.tile([128, FREE], F32, tag="t1")
                    for c in range(C):
                        v00 = gt[:, :, 0 + c]
                        v01 = gt[:, :, 3 + c]
                        v10 = gt[:, :, 6 + c]
                        v11 = gt[:, :, 9 + c]
                        nc.vector.tensor_tensor(out=t0[:], in0=v01, in1=v00, op=ALU.subtract)
                        nc.vector.tensor_tensor(out=t0[:], in0=t0[:], in1=fx[:], op=ALU.mult)
                        nc.vector.tensor_tensor(out=t0[:], in0=t0[:], in1=v00, op=ALU.add)
                        nc.vector.tensor_tensor(out=t1[:], in0=v11, in1=v10, op=ALU.subtract)
                        nc.vector.tensor_tensor(out=t1[:], in0=t1[:], in1=fx[:], op=ALU.mult)
                        nc.vector.tensor_tensor(out=t1[:], in0=t1[:], in1=v10, op=ALU.add)
                        nc.vector.tensor_tensor(out=t0[:], in0=t0[:], in1=bv[:], op=ALU.mult)
                        nc.vector.tensor_tensor(out=t1[:], in0=t1[:], in1=av[:], op=ALU.mult)
                        nc.vector.tensor_tensor(out=t0[:], in0=t0[:], in1=t1[:], op=ALU.add)
                        nc.vector.tensor_tensor(out=accs[c][:], in0=accs[c][:],
                                                in1=t0[:], op=ALU.add)

                for c in range(C):
                    nc.sync.dma_start(
                        out=out[c, blk * BLK:(blk + 1) * BLK, :].rearrange(
                            "y (xh p) -> p (y xh)", p=128),
                        in_=accs[c][:])
    nc.compile()
    return nc


_CACHED = {}


def kernel(image, offsets, weights):
    image = np.ascontiguousarray(image, dtype=np.float32)
    offsets = np.ascontiguousarray(offsets, dtype=np.float32)
    weights = np.ascontiguousarray(weights, dtype=np.float32)
    assert image.shape == (B, C, H, W)
    m = float(np.abs(offsets).max())
    assert m * C1 + 0.5 < PAD, f"offset magnitude {m} exceeds halo budget"

    if "nc" not in _CACHED:
        _CACHED["nc"] = build_program()
    nc = _CACHED["nc"]

    xs = np.arange(W, dtype=np.float64)
    # xgrid[p, f]: x = (f%2)*128 + p
    xgrid = np.zeros((128, FREE), dtype=np.float32)
    for xh in range(2):
        xgrid[:, xh::2] = (xs[xh * 128:(xh + 1) * 128] * C1 - 0.5)[:, None]

    in_maps = []
    for core in range(8):
        b, h = core // 2, core % 2
        b0v = 128 * h - PAD
        rows = np.clip(np.arange(b0v, b0v + SLAB), 0, H - 1)
        img_slab = image[b][:, rows, :]
        offs = offsets[b][:, 128 * h:128 * h + 128, :]
        wts = weights[b][:, 128 * h:128 * h + 128, :]
        ygrid = np.zeros((2, 128, FREE), dtype=np.float32)
        for blk in range(2):
            ys = 128 * h + blk * BLK + np.arange(BLK, dtype=np.float64)
            vals = ys * C1 - 0.5 - b0v
            for xh in range(2):
                ygrid[blk, :, xh::2] = vals[None, :].astype(np.float32)
        cconst = np.zeros((128, 2), dtype=np.float32)
        cconst[:, 0] = 0.0 - b0v
        cconst[:, 1] = (H - 1) - b0v
        in_maps.append({
            "img": np.ascontiguousarray(img_slab),
            "offs": np.ascontiguousarray(offs),
            "wts": np.ascontiguousarray(wts),
            "ygrid": ygrid, "xgrid": xgrid, "cconst": cconst,
        })

    res = run_bass_kernel_spmd(nc, in_maps, core_ids=list(range(8)))
    _CACHED["last_results"] = res

    out = np.zeros((B, C, H, W), dtype=np.float32)
    for core in range(8):
        b, h = core // 2, core % 2
        out[b][:, 128 * h:128 * h + 128, :] = res.results[core]["out"]
    return out
